# revision 28
# baseline (speedup 1.0000x reference)
"""Trainium2 Bass kernel for nn_MoEClassifier (6-layer transformer backbone +
softmax-routed MoE head), SPMD over 8 NeuronCores.

Sharding: data-parallel backbone (2 of 16 batch rows per core, params
replicated), expert-parallel MoE head (core c owns expert c) glued by an
on-device AllGather of the pooled features; the host sums the 8 per-expert
partial outputs.

v2: activations + weights in bf16 (fp32 PSUM accumulation), one
program-lifetime TileContext with persistent PSUM pools (8 banks: 4 "mm"
rotating accumulators, 2 "po", 2 "stat") so phases pipeline across the
layer instead of serializing at per-phase pool boundaries.  Weights are
host-pre-transposed into per-partition-contiguous bf16 blocks (1 DMA per
layer for Wqkv/Wo, chunked double-buffered W1/W2).  The LN rstd comes from
a raw Rsqrt activation (eps folded into its bias); LN2 of each half is
emitted right after that half's Wo, and each half's LN1-of-next-layer is
emitted right after its W2, so every LN chain hides under the other
half's compute.  Attention softmax denominators are batched four heads
per DVE reciprocal (at partitions 0/32/64/96 so the K=1 re-broadcast
matmuls stay legal), with the broadcast matmuls allocated after all score
tiles so a blocked broadcast never plugs the PSUM "mm" rotation.  W2
accumulates per-(fg, m) PSUM partials into the residual
with DVE adds so no PSUM bank is held across the whole FFN.  The MoE head
runs token-major (N=512 matmuls) with expert weights DMA'd into SBUF space
freed by the backbone weight pools while the AllGather is in flight."""

import numpy as np
import ml_dtypes

import concourse.bass as bass
import concourse.mybir as mybir
from concourse.bass_utils import run_bass_kernel_spmd
from concourse.tile import TileContext
from concourse.vector_clock import ScopedClock

B, S, V, H, L, NH, FF, E, FE, C = 16, 512, 30522, 768, 6, 8, 3072, 8, 3072, 1000
HD = H // NH          # 96
NCORES = 8
BL = B // NCORES      # 2 batch rows per core
T = BL * S            # 1024 tokens per core
HC = H // 128         # 6 hidden chunks
FFC = FF // 128       # 24 ffn chunks
EPS = 1e-5

f32 = mybir.dt.float32
f32r = mybir.dt.float32r
bf16 = mybir.dt.bfloat16
AF = mybir.ActivationFunctionType
AX = mybir.AxisListType
OP = mybir.AluOpType
ts = bass.ts

MAX_WAITS = 1

BF = ml_dtypes.bfloat16


class PatchedTileContext(TileContext):
    """Workaround for this walrus build's 1-sync-wait-per-instruction limit:
    split excess semaphore waits onto single-wait NOPs inserted immediately
    before the owning instruction (same engine, same program point)."""

    def _split_excess_waits(self, ordered):
        nc = self.nc
        for bb_name, insts in list(ordered.items()):
            new_list = []
            changed = False
            for inst in insts:
                si = getattr(inst, "sync_info", None)
                if si is not None and len(si.on_wait) > MAX_WAITS:
                    waits = list(si.on_wait)
                    movable = [
                        w for w in waits
                        if w.sync_type == "semaphore" and w.wait_mode == "sem-ge-imm"
                    ]
                    n_fixed = len(waits) - len(movable)
                    keep_n = max(0, MAX_WAITS - n_fixed)
                    n_over = max(0, len(movable) - keep_n)
                    overflow = movable[:n_over]
                    keep = [w for w in waits if w not in overflow]
                    assert len(keep) <= MAX_WAITS, (
                        f"cannot legalize waits on {inst.name}"
                    )
                    for w in overflow:
                        nop = mybir.InstNoOp(
                            name=f"I-{nc.next_id()}",
                            sync_info=mybir.SyncInfo(on_wait=[w], on_update=[]),
                            bass_nofuse=True,
                            engine=inst.engine,
                        )
                        new_list.append(nop)
                    inst.sync_info = mybir.SyncInfo(
                        on_wait=keep, on_update=list(si.on_update)
                    )
                    changed = True
                new_list.append(inst)
            if changed:
                ordered[bb_name] = new_list

    def _lower_ordered_insts(self, ordered):
        self._split_excess_waits(ordered)
        return super()._lower_ordered_insts(ordered)

    def _drain_and_barrier(self, tick_clock, wait_clock):
        nops = [self.nc.sync.nop(nofuse=True, hint=f"dw_{i}") for i in range(40)]
        drain_inst = self.nc.sync.drain()
        wait_clock.add_sem_waits(
            drain_inst.ins, ScopedClock({None: tick_clock.global_clock})
        )
        si = drain_inst.ins.sync_info
        if si is not None and len(si.on_wait) > 1:
            waits = list(si.on_wait)
            rest, keep = waits[:-1], waits[-1:]
            assert len(rest) <= len(nops)
            for nop_bi, w in zip(nops, rest):
                nop_bi.ins.sync_info = mybir.SyncInfo(on_wait=[w], on_update=[])
            drain_inst.ins.sync_info = mybir.SyncInfo(
                on_wait=keep, on_update=list(si.on_update)
            )
        self.nc.all_engine_barrier()
        assert self.sems is not None
        popped = self.nc._tile_sem_poison_stack.pop()
        assert popped is self._sem_poison
        self.nc.clear_and_free_semaphores(list(self.sems.allocated().values()))
        self.nc.all_engine_barrier()


def _r(ap):
    return ap.bitcast(f32r)


def _act_raw(nc, out, in_, func, bias=None):
    """scalar.activation without bass's Reciprocal/Rsqrt accuracy guard.
    out = func(in_ + bias); scale=1."""
    if bias is None:
        bias = nc.const_aps.scalar_like(0.0, in_)
    eng = nc.scalar
    ins = [eng.lower_ap(in_), eng.lower_ap(bias),
           mybir.ImmediateValue(dtype=f32, value=1.0),
           mybir.ImmediateValue(dtype=f32, value=0.0)]
    return eng.add_instruction(
        mybir.InstActivation(
            name=nc.get_next_instruction_name(),
            func=func,
            ins=ins,
            outs=[eng.lower_ap(out)],
        )
    )


def build_program(n_layers=L, debug=False):
    nc = bass.Bass()

    x0T_d = nc.dram_tensor("x0T", [128, HC, T], bf16, kind="ExternalInput")
    wqkv_d = nc.dram_tensor("wqkvT", [n_layers, 128, HC, 3 * H], bf16,
                            kind="ExternalInput")
    wo_d = nc.dram_tensor("woT", [n_layers, HD, NH, H], bf16,
                          kind="ExternalInput")
    w1_d = nc.dram_tensor("w1T", [n_layers, 128, HC, FF], bf16,
                          kind="ExternalInput")
    w2_d = nc.dram_tensor("w2T", [n_layers, 128, FFC, H], bf16,
                          kind="ExternalInput")
    wr_d = nc.dram_tensor("wrT", [128, HC, E], bf16, kind="ExternalInput")
    we1_d = nc.dram_tensor("we1T", [128, HC, FE], bf16, kind="ExternalInput")
    we2_d = nc.dram_tensor("we2T", [128, FFC, C], bf16, kind="ExternalInput")
    maske_d = nc.dram_tensor("maske", [B, E], f32, kind="ExternalInput")
    ones_d = nc.dram_tensor("ones", [128, 128], f32, kind="ExternalInput")
    onesb_d = nc.dram_tensor("onesb", [128, 128], bf16, kind="ExternalInput")
    id128_d = nc.dram_tensor("id128", [128, 128], f32, kind="ExternalInput")
    id16_d = nc.dram_tensor("id16", [16, 16], f32, kind="ExternalInput")
    y_d = nc.dram_tensor("y", [B, C], f32, kind="ExternalOutput")
    cc_in = nc.dram_tensor("cc_in", [BL, H], f32)
    cc_out = nc.dram_tensor("cc_out", [B, H], f32, addr_space="Shared")

    lp = nc.allow_low_precision(reason="bf16 activations/weights by design")
    lp.__enter__()
    with PatchedTileContext(nc) as tc:
        with tc.tile_pool(name="const", bufs=1) as cpool, \
             tc.tile_pool(name="act", bufs=1) as act, \
             tc.tile_pool(name="rows", bufs=2) as rows, \
             tc.tile_pool(name="psmm", bufs=4, space="PSUM") as psmm, \
             tc.tile_pool(name="pspo", bufs=2, space="PSUM") as pspo, \
             tc.tile_pool(name="psst", bufs=2, space="PSUM") as psst:

            # ---------------- constants
            onescol = cpool.tile([128, 1], bf16, tag="onescol")
            nc.sync.dma_start(onescol[:], onesb_d[:, 0:1])
            onesrow = cpool.tile([1, 128], bf16, tag="onesrow")
            nc.sync.dma_start(onesrow[:], onesb_d[0:1, :])
            id16 = cpool.tile([16, 16], f32, tag="id16")
            nc.sync.dma_start(id16[:], id16_d[:])
            id128 = cpool.tile([128, 128], f32, tag="id128")
            nc.sync.dma_start(id128[:], id128_d[:])
            ones128b = cpool.tile([128, 128], bf16, tag="ones128b")
            nc.sync.dma_start(ones128b[:], onesb_d[:])
            eps_row = cpool.tile([1, 1], f32, tag="eps_row")
            nc.vector.memset(eps_row[:], EPS)

            # ---------------- persistent activations (bf16)
            x = act.tile([128, HC, T], bf16, tag="x")
            nc.sync.dma_start(x[:, :, 0:512], x0T_d[:, :, 0:512])
            nc.sync.dma_start(x[:, :, 512:1024], x0T_d[:, :, 512:1024])
            hT = act.tile([128, HC, T], bf16, tag="hT")
            hT2 = act.tile([128, HC, T], bf16, tag="hT2")
            qT = act.tile([HD, NH, 512], bf16, tag="qT")
            kT = act.tile([HD, NH, 512], bf16, tag="kT")
            oT = act.tile([HD, NH, 512], bf16, tag="oT")
            pooledT = act.tile([128, HC, BL], f32, tag="pooledT")
            pool_tok = act.tile([BL, H], f32, tag="pool_tok")

            def layer_norm_half(bca, xsrc, hdst, tq, name):
                """hdst[:, :, tq*512:+512] = LN(xsrc same region). Stats via
                PE ones-matmuls, chain on DVE/scalar, broadcasts via PE."""
                tqs = ts(tq, 512)
                # s1 in array col-group 0, s2 in col-group 1: the pair
                # runs concurrently in the PE (separate output strips/XBUS)
                s1 = psst.tile([1, 512], f32, tag="stat", name=f"s1_{name}")
                s2t = psst.tile([33, 512], f32, tag="stat",
                                name=f"s2_{name}")
                s2 = s2t[32:33, :]
                for hc in range(HC):
                    sq = bca.tile([128, 512], bf16, tag="sq",
                                  name=f"sq_{name}_{hc}")
                    nc.scalar.activation(sq[:], xsrc[:, hc, tqs], AF.Square)
                    nc.tensor.matmul(s1[:], onescol[:], xsrc[:, hc, tqs],
                                     start=(hc == 0), stop=(hc == HC - 1),
                                     tile_position=(0, 0))
                    nc.tensor.matmul(s2, onescol[:], sq[:],
                                     start=(hc == 0), stop=(hc == HC - 1),
                                     tile_position=(0, 32))
                mu = rows.tile([1, 512], f32, tag="mu", name=f"mu_{name}")
                var = rows.tile([1, 512], f32, tag="var", name=f"var_{name}")
                rstd = rows.tile([1, 512], f32, tag="rstd",
                                 name=f"rstd_{name}")
                nbr = rows.tile([1, 512], f32, tag="nbr", name=f"nbr_{name}")
                msq = rows.tile([1, 512], f32, tag="msq", name=f"msq_{name}")
                nc.vector.tensor_scalar_mul(mu[:], s1[:], 1.0 / H)
                nc.vector.tensor_scalar_mul(var[:], s2[:], 1.0 / H)
                nc.vector.tensor_tensor(msq[:], mu[:], mu[:], OP.mult)
                nc.vector.tensor_tensor(var[:], var[:], msq[:], OP.subtract)
                _act_raw(nc, rstd[:], var[:], AF.Rsqrt, bias=eps_row[:])
                # nbr = -mu * rstd  (so hT = x*rb + nbr_bcast)
                nc.vector.tensor_tensor(nbr[:], mu[:], rstd[:], OP.mult)
                nbr_b = rows.tile([1, 512], bf16, tag="nbr_b",
                                  name=f"nbr_b_{name}")
                rstd_b = rows.tile([1, 512], bf16, tag="rstd_b",
                                   name=f"rstd_b_{name}")
                nc.vector.tensor_scalar_mul(nbr_b[:], nbr[:], -1.0)
                nc.scalar.copy(rstd_b[:], rstd[:])
                prb = psmm.tile([128, 512], f32, tag="mm", name=f"rb_{name}")
                pnb = psmm.tile([128, 512], f32, tag="mm", name=f"nb_{name}")
                nc.tensor.matmul(prb[:], onesrow[:], rstd_b[:],
                                 start=True, stop=True)
                nc.tensor.matmul(pnb[:], onesrow[:], nbr_b[:],
                                 start=True, stop=True)
                rb_b = bca.tile([128, 512], bf16, tag="rb_b",
                                name=f"rb_b_{name}")
                nb_b = bca.tile([128, 512], bf16, tag="nb_b",
                                name=f"nb_b_{name}")
                nc.scalar.copy(rb_b[:], prb[:])
                nc.scalar.copy(nb_b[:], pnb[:])
                for hc in range(HC):
                    t1 = bca.tile([128, 512], bf16, tag="lnt",
                                  name=f"lnt_{name}_{hc}")
                    nc.vector.tensor_tensor(t1[:], xsrc[:, hc, tqs], rb_b[:],
                                            OP.mult)
                    nc.vector.tensor_tensor(hdst[:, hc, tqs], t1[:], nb_b[:],
                                            OP.add)

            with tc.tile_pool(name="bca", bufs=2) as bca, \
                 tc.tile_pool(name="attn", bufs=2) as attn, \
                 tc.tile_pool(name="fft", bufs=3) as fftp, \
                 tc.tile_pool(name="wqkv", bufs=1) as wqkvp, \
                 tc.tile_pool(name="wo", bufs=1) as wop, \
                 tc.tile_pool(name="w1", bufs=2) as w1p, \
                 tc.tile_pool(name="w2", bufs=2) as w2p:

                # LN1 for layer 0 (later layers' LN1 is emitted inside
                # the previous layer's FFN so it hides under the other half)
                for tq in range(2):
                    layer_norm_half(bca, x, hT, tq, f"ln1_0_{tq}")

                for l in range(n_layers):
                    # ---------------- layer weights (1 DMA each for qkv/wo)
                    wqkv_sb = wqkvp.tile([128, HC, 3 * H], bf16, tag="wqkv",
                                         name=f"wqkv_{l}")
                    nc.sync.dma_start(wqkv_sb[:], wqkv_d[l])
                    wo_sb = wop.tile([HD, NH, H], bf16, tag="wo",
                                     name=f"wo_{l}")
                    nc.sync.dma_start(wo_sb[:], wo_d[l])

                    # ---------------- attention per batch row (=token half)
                    for b2 in range(BL):
                        tqs = ts(b2, 512)
                        # QKV projections
                        # 128-wide weight slices (not 96) so FWL stays
                        # enabled; rows 96-127 of the psum are next-head
                        # garbage, dropped at evacuation
                        for h in range(NH):
                            pq = psmm.tile([128, 512], f32, tag="mm",
                                           name=f"pq_{l}_{b2}_{h}")
                            pk = psmm.tile([128, 512], f32, tag="mm",
                                           name=f"pk_{l}_{b2}_{h}")
                            for hc in range(HC):
                                rhs = hT[:, hc, tqs]
                                nc.tensor.matmul(
                                    pq[:], wqkv_sb[:, hc, h * HD:h * HD + 128],
                                    rhs, start=(hc == 0), stop=(hc == HC - 1))
                                nc.tensor.matmul(
                                    pk[:],
                                    wqkv_sb[:, hc,
                                            H + h * HD:H + h * HD + 128],
                                    rhs, start=(hc == 0), stop=(hc == HC - 1))
                            nc.any.tensor_copy(qT[:, h, :], pq[:HD, :])
                            nc.any.tensor_copy(kT[:, h, :], pk[:HD, :])
                        # V (token-major, ones-augmented for softmax denom)
                        v_augf = attn.tile([128, 4 * NH * (HD + 1) + 32],
                                           bf16, tag="vaug",
                                           name=f"vaug_{l}_{b2}")
                        v_aug = v_augf[:, :4 * NH * (HD + 1)].rearrange(
                            "p (tk h d) -> p tk h d", tk=4, h=NH)
                        nc.vector.memset(v_aug[:, :, :, HD:], 1.0)
                        nc.vector.memset(
                            v_augf[:, 4 * NH * (HD + 1):], 0.0)
                        for n2 in range(2):
                            for tt in range(4):
                                pv = psmm.tile([128, 384], f32, tag="mm",
                                               name=f"pv_{l}_{b2}_{n2}_{tt}")
                                for hc in range(HC):
                                    lhs = hT[:, hc, b2 * 512 + tt * 128:
                                             b2 * 512 + (tt + 1) * 128]
                                    nc.tensor.matmul(
                                        pv[:], lhs,
                                        wqkv_sb[:, hc,
                                                2 * H + n2 * 384:
                                                2 * H + (n2 + 1) * 384],
                                        start=(hc == 0), stop=(hc == HC - 1))
                                dst = v_aug[:, tt, n2 * 4:(n2 + 1) * 4, :HD]
                                nc.any.tensor_copy(
                                    dst,
                                    pv[:].rearrange("p (h d) -> p h d", h=4))
                        # scores -> softmax -> AV for all heads; denominators
                        # batched 4 heads per DVE reciprocal (rows 0/32/64/96)
                        dns = []
                        rcps = []
                        for hb in range(2):
                            dn = rows.tile([128, 512], f32, tag="dn",
                                           name=f"dn_{l}_{b2}_{hb}")
                            nc.vector.memset(dn[:], 1.0)
                            dns.append(dn)
                        for h in range(NH):
                            hb, h4 = divmod(h, 4)
                            expT = attn.tile([128, 4, 512], bf16,
                                             tag="expT",
                                             name=f"expT_{l}_{b2}_{h}")
                            for tk in range(4):
                                psc = psmm.tile(
                                    [128, 512], f32, tag="mm",
                                    name=f"sc_{l}_{b2}_{h}_{tk}")
                                nc.tensor.matmul(
                                    psc[:], kT[:, h, ts(tk, 128)],
                                    qT[:, h, :], start=True, stop=True)
                                nc.scalar.activation(
                                    expT[:, tk, :], psc[:], AF.Exp,
                                    scale=float(1.0 / np.sqrt(HD)))
                            po = pspo.tile([128, 512], f32, tag="po",
                                           name=f"po_{l}_{b2}_{h}")
                            for tk in range(4):
                                off = (tk * NH + h) * (HD + 1)
                                nc.tensor.matmul(po[:],
                                                 v_augf[:, off:off + 128],
                                                 expT[:, tk, :],
                                                 start=(tk == 0),
                                                 stop=(tk == 3))
                            nc.any.tensor_copy(oT[:, h, :], po[:HD, :])
                            nc.any.tensor_copy(
                                dns[hb][32 * h4:32 * h4 + 1, :],
                                po[HD:HD + 1, :])
                            if h4 == 3:
                                rcp_bf = rows.tile([128, 512], bf16,
                                                   tag="rcp_bf",
                                                   name=f"rcpbf_{l}_{b2}_{hb}")
                                nc.vector.reciprocal(rcp_bf[:], dns[hb][:])
                                rcps.append(rcp_bf)
                        # normalize (prb tiles allocated after all psc tiles
                        # so blocked broadcasts don't plug the mm rotation)
                        for h in range(NH):
                            hb, h4 = divmod(h, 4)
                            prb = psmm.tile([128, 512], f32, tag="mm",
                                            name=f"prb_{l}_{b2}_{h}")
                            nc.tensor.matmul(
                                prb[:],
                                ones128b[32 * h4:32 * h4 + 1, :],
                                rcps[hb][32 * h4:32 * h4 + 1, :],
                                start=True, stop=True,
                                tile_position=(32 * h4, 0))
                            rb_at = attn.tile([HD, 512], bf16,
                                              tag="rb_at",
                                              name=f"rbat_{l}_{b2}_{h}")
                            nc.scalar.copy(rb_at[:], prb[:HD, :])
                            nc.vector.tensor_tensor(oT[:, h, :],
                                                    oT[:, h, :],
                                                    rb_at[:], OP.mult)
                        # Wo + residual
                        for m in range(HC):
                            pwo = pspo.tile([128, 512], f32, tag="po",
                                            name=f"pwo_{l}_{b2}_{m}")
                            for h in range(NH):
                                nc.tensor.matmul(pwo[:],
                                                 wo_sb[:, h, ts(m, 128)],
                                                 oT[:, h, :],
                                                 start=(h == 0),
                                                 stop=(h == NH - 1))
                            nc.vector.tensor_tensor(x[:, m, tqs],
                                                    x[:, m, tqs], pwo[:],
                                                    OP.add)
                        # LN2 for this half right away (overlaps other half's
                        # attention)
                        layer_norm_half(bca, x, hT2, b2, f"ln2_{l}_{b2}")

                    # ---------------- FFN per token half; each half's output
                    # feeds the next layer's LN1 (or the final LN) immediately
                    for tq in range(2):
                        tqs = ts(tq, 512)
                        ffT_tiles = {}
                        for fg in range(6):
                            w1g = w1p.tile([128, HC, 512], bf16, tag="w1",
                                           name=f"w1_{l}_{tq}_{fg}")
                            nc.sync.dma_start(
                                w1g[:], w1_d[l, :, :, ts(fg, 512)])
                            ffT = fftp.tile([128, 4, 512], bf16, tag="ffT",
                                            name=f"ffT_{l}_{tq}_{fg}")
                            ffT_tiles[(tq, fg)] = ffT
                            for ff in range(4):
                                pf = psmm.tile([128, 512], f32, tag="mm",
                                               name=f"pf_{l}_{tq}_{fg}_{ff}")
                                for hc in range(HC):
                                    nc.tensor.matmul(
                                        pf[:],
                                        w1g[:, hc, ts(ff, 128)],
                                        hT2[:, hc, tqs],
                                        start=(hc == 0), stop=(hc == HC - 1))
                                nc.scalar.activation(ffT[:, ff, :], pf[:],
                                                     AF.Gelu)
                        for fg in range(6):
                            w2g = w2p.tile([128, 4, H], bf16, tag="w2",
                                           name=f"w2_{l}_{tq}_{fg}")
                            nc.sync.dma_start(
                                w2g[:], w2_d[l, :, fg * 4:(fg + 1) * 4, :])
                            ffTg = ffT_tiles[(tq, fg)]
                            for m in range(HC):
                                px = pspo.tile([128, 512], f32, tag="po",
                                               name=f"px_{l}_{tq}_{fg}_{m}")
                                for ff in range(4):
                                    nc.tensor.matmul(
                                        px[:], w2g[:, ff, ts(m, 128)],
                                        ffTg[:, ff, :],
                                        start=(ff == 0), stop=(ff == 3))
                                nc.vector.tensor_tensor(x[:, m, tqs],
                                                        x[:, m, tqs], px[:],
                                                        OP.add)
                        if l < n_layers - 1:
                            layer_norm_half(bca, x, hT, tq,
                                            f"ln1_{l + 1}_{tq}")
                        else:
                            layer_norm_half(bca, x, hT, tq, f"lnf_{tq}")
                            acc = rows.tile([128, HC, 1], f32, tag="poolacc",
                                            name=f"poolacc_{tq}")
                            nc.vector.reduce_sum(acc[:],
                                                 hT[:, :, ts(tq, 512)],
                                                 axis=AX.X)
                            nc.vector.tensor_scalar_mul(
                                pooledT[:, :, tq:tq + 1], acc[:], 1.0 / S)

                # -------------- pooled transpose + AllGather
                for hc in range(HC):
                    pt = psmm.tile([BL, 128], f32, tag="mm",
                                   name=f"ptr_{hc}")
                    nc.tensor.transpose(pt[:], pooledT[:, hc, :], id128[:])
                    nc.any.tensor_copy(pool_tok[:, ts(hc, 128)], pt[:])
                nc.gpsimd.dma_start(cc_in[:], pool_tok[:])
                nc.gpsimd.collective_compute(
                    "AllGather", OP.bypass,
                    replica_groups=[list(range(NCORES))],
                    ins=[cc_in[:]], outs=[cc_out[:]],
                )

            # -------------- MoE head (expert-parallel).  The backbone
            # weight/attn pools are closed, so this pool reuses their SBUF and
            # the expert-weight DMAs start as soon as the last readers retire.
            with tc.tile_pool(name="head", bufs=1) as hp:
                we1_sb = hp.tile([128, HC, FE], bf16, tag="we1")
                nc.sync.dma_start(we1_sb[:], we1_d[:])
                we2_sb = hp.tile([128, FFC, C], bf16, tag="we2")
                nc.sync.dma_start(we2_sb[:], we2_d[:])
                wr_sb = hp.tile([128, HC, E], bf16, tag="wr")
                nc.sync.dma_start(wr_sb[:], wr_d[:])
                maske = hp.tile([B, E], f32, tag="maske")
                nc.sync.dma_start(maske[:], maske_d[:])
                id16b = hp.tile([16, 16], bf16, tag="id16b")
                nc.any.tensor_copy(id16b[:], id16[:])

                pg = hp.tile([B, H], f32, tag="pg")
                nc.gpsimd.dma_start(pg[:], cc_out[:])
                paT = hp.tile([128, HC, B], bf16, tag="paT")
                for hc in range(HC):
                    ptr = psmm.tile([128, B], f32, tag="mm",
                                    name=f"hptr_{hc}")
                    nc.tensor.transpose(ptr[:], pg[:, ts(hc, 128)], id16[:])
                    nc.any.tensor_copy(paT[:, hc, :], ptr[:])
                # gate
                pgl = psst.tile([B, E], f32, tag="stat", name="pgl")
                for hc in range(HC):
                    nc.tensor.matmul(pgl[:], paT[:, hc, :], wr_sb[:, hc, :],
                                     start=(hc == 0), stop=(hc == HC - 1))
                gate = hp.tile([B, E], f32, tag="gate")
                gmax = rows.tile([B, 1], f32, tag="grow")
                nc.vector.reduce_max(gmax[:], pgl[:], axis=AX.X)
                ngmax = rows.tile([B, 1], f32, tag="grow2")
                nc.vector.tensor_scalar_mul(ngmax[:], gmax[:], -1.0)
                nc.scalar.activation(gate[:], pgl[:], AF.Exp, bias=ngmax[:])
                gsum = rows.tile([B, 1], f32, tag="grow3")
                nc.vector.reduce_sum(gsum[:], gate[:], axis=AX.X)
                grecip = rows.tile([B, 1], f32, tag="grow4")
                nc.vector.reciprocal(grecip[:], gsum[:])
                nc.vector.tensor_scalar_mul(gate[:], gate[:], grecip[:])
                gcol = hp.tile([B, 1], f32, tag="gcol")
                nc.vector.tensor_tensor(maske[:], gate[:], maske[:], OP.mult)
                nc.vector.reduce_sum(gcol[:], maske[:], axis=AX.X)

                # expert hidden, token-major [B, FE]
                eh_tok = hp.tile([B, FE], bf16, tag="eh_tok")
                for fet in range(6):
                    pe_ = psmm.tile([B, 512], f32, tag="mm",
                                    name=f"pe_{fet}")
                    for hc in range(HC):
                        nc.tensor.matmul(pe_[:], paT[:, hc, :],
                                         we1_sb[:, hc, ts(fet, 512)],
                                         start=(hc == 0), stop=(hc == HC - 1))
                    nc.scalar.activation(eh_tok[:, ts(fet, 512)], pe_[:],
                                         AF.Gelu)
                # transpose to feature-major [FE, B]
                ehT = hp.tile([128, FFC, B], bf16, tag="ehT")
                for fc in range(FFC):
                    ptb = psmm.tile([128, B], bf16, tag="mm",
                                    name=f"ptb_{fc}")
                    nc.tensor.transpose(ptb[:], eh_tok[:, ts(fc, 128)],
                                        id16b[:])
                    nc.any.tensor_copy(ehT[:, fc, :], ptb[:])
                # expert logits, scaled by this expert's gate column
                y_sb = hp.tile([B, C], f32, tag="y")
                for cn in range(2):
                    csz = C // 2
                    pel = pspo.tile([B, csz], f32, tag="po",
                                    name=f"pel_{cn}")
                    for fc in range(FFC):
                        nc.tensor.matmul(pel[:], ehT[:, fc, :],
                                         we2_sb[:, fc, ts(cn, csz)],
                                         start=(fc == 0), stop=(fc == FFC - 1))
                    nc.vector.tensor_scalar_mul(y_sb[:, ts(cn, csz)], pel[:],
                                                gcol[:])
                nc.sync.dma_start(y_d[:], y_sb[:])

    lp.__exit__(None, None, None)
    return nc, {}


_CACHE = {}


def _get_program(n_layers=L, debug=False):
    key = (n_layers, debug)
    if key not in _CACHE:
        _CACHE[key] = build_program(n_layers, debug)
    return _CACHE[key]


def prepare_inputs(inputs, n_layers=L):
    """Host-side shard prep: embedding gather, bf16 weight transposes,
    per-core slicing, asserts."""
    ids = np.asarray(inputs["input_ids"])
    mask = np.asarray(inputs["attention_mask"])
    assert (mask == 1).all(), "kernel assumes attention_mask == ones"
    for k in ("bqkv", "bo", "b1", "b2", "br", "be1", "be2",
              "ln1_b", "ln2_b", "lnf_b"):
        assert not np.any(np.asarray(inputs[k])), f"{k} must be zero"
    for k in ("ln1_g", "ln2_g", "lnf_g"):
        assert np.all(np.asarray(inputs[k]) == 1.0), f"{k} must be ones"

    tok = np.asarray(inputs["tok_emb"], np.float32)
    pos = np.asarray(inputs["pos_emb"], np.float32)
    x0 = tok[ids] + pos[None]                      # [B, S, H]

    wqkv = np.asarray(inputs["Wqkv"], np.float32)[:n_layers]   # [L,H,3H]
    wqkvT = np.ascontiguousarray(
        wqkv.reshape(n_layers, HC, 128, 3 * H).transpose(0, 2, 1, 3)
    ).astype(BF)                                               # [L,128,HC,3H]
    wo = np.asarray(inputs["Wo"], np.float32)[:n_layers]       # [L,H,H]
    woT = np.ascontiguousarray(
        wo.reshape(n_layers, NH, HD, H).transpose(0, 2, 1, 3)
    ).astype(BF)                                               # [L,HD,NH,H]
    w1 = np.asarray(inputs["W1"], np.float32)[:n_layers]       # [L,H,FF]
    w1T = np.ascontiguousarray(
        w1.reshape(n_layers, HC, 128, FF).transpose(0, 2, 1, 3)
    ).astype(BF)                                               # [L,128,HC,FF]
    w2 = np.asarray(inputs["W2"], np.float32)[:n_layers]       # [L,FF,H]
    w2T = np.ascontiguousarray(
        w2.reshape(n_layers, FFC, 128, H).transpose(0, 2, 1, 3)
    ).astype(BF)                                               # [L,128,FFC,H]
    wr = np.asarray(inputs["Wr"], np.float32)                  # [H,E]
    wrT = np.ascontiguousarray(
        wr.reshape(HC, 128, E).transpose(1, 0, 2)).astype(BF)  # [128,HC,E]
    we1 = np.asarray(inputs["We1"], np.float32)                # [E,H,FE]
    we2 = np.asarray(inputs["We2"], np.float32)                # [E,FE,C]
    id16 = np.eye(16, dtype=np.float32)
    id128 = np.eye(128, dtype=np.float32)
    ones = np.ones((128, 128), np.float32)

    in_maps = []
    for c in range(NCORES):
        rows_ = x0[c * BL:(c + 1) * BL]             # [BL, S, H]
        x0T = rows_.reshape(T, H).T                 # [H, T]
        x0Tr = np.ascontiguousarray(
            x0T.reshape(HC, 128, T).transpose(1, 0, 2)).astype(BF)
        maske = np.zeros((B, E), np.float32)
        maske[:, c] = 1.0
        we1T = np.ascontiguousarray(
            we1[c].reshape(HC, 128, FE).transpose(1, 0, 2)).astype(BF)
        we2T = np.ascontiguousarray(
            we2[c].reshape(FFC, 128, C).transpose(1, 0, 2)).astype(BF)
        in_maps.append({
            "x0T": x0Tr, "wqkvT": wqkvT, "woT": woT, "w1T": w1T, "w2T": w2T,
            "wrT": wrT, "we1T": we1T, "we2T": we2T,
            "maske": maske, "ones": ones,
            "onesb": ones.astype(BF), "id128": id128, "id16": id16,
        })
    return in_maps


def kernel(**inputs):
    nc, _dbg = _get_program(L, debug=False)
    in_maps = prepare_inputs(inputs, L)
    res = run_bass_kernel_spmd(nc, in_maps, core_ids=list(range(NCORES)))
    out = np.zeros((B, C), np.float32)
    for r_ in res.results:
        out += r_["y"]
    return out


# revision 29
# speedup vs baseline: 1.0054x; 1.0054x over previous
"""Trainium2 Bass kernel for nn_MoEClassifier (6-layer transformer backbone +
softmax-routed MoE head), SPMD over 8 NeuronCores.

Sharding: data-parallel backbone (2 of 16 batch rows per core, params
replicated), expert-parallel MoE head (core c owns expert c) glued by an
on-device AllGather of the pooled features; the host sums the 8 per-expert
partial outputs.

v2: activations + weights in bf16 (fp32 PSUM accumulation), one
program-lifetime TileContext with persistent PSUM pools (8 banks: 4 "mm"
rotating accumulators, 2 "po", 2 "stat") so phases pipeline across the
layer instead of serializing at per-phase pool boundaries.  Weights are
host-pre-transposed into per-partition-contiguous bf16 blocks (1 DMA per
layer for Wqkv/Wo, chunked double-buffered W1/W2).  The LN rstd comes from
a raw Rsqrt activation (eps folded into its bias); LN2 of each half is
emitted right after that half's Wo, and each half's LN1-of-next-layer is
emitted right after its W2, so every LN chain hides under the other
half's compute.  Attention softmax denominators are batched four heads
per DVE reciprocal (at partitions 0/32/64/96 so the K=1 re-broadcast
matmuls stay legal), with the broadcast matmuls allocated after all score
tiles so a blocked broadcast never plugs the PSUM "mm" rotation.  W2
accumulates per-(fg, m) PSUM partials into the residual
with DVE adds so no PSUM bank is held across the whole FFN.  The MoE head
runs token-major (N=512 matmuls) with expert weights DMA'd into SBUF space
freed by the backbone weight pools while the AllGather is in flight."""

import numpy as np
import ml_dtypes

import concourse.bass as bass
import concourse.mybir as mybir
from concourse.bass_utils import run_bass_kernel_spmd
from concourse.tile import TileContext
from concourse.vector_clock import ScopedClock

B, S, V, H, L, NH, FF, E, FE, C = 16, 512, 30522, 768, 6, 8, 3072, 8, 3072, 1000
HD = H // NH          # 96
NCORES = 8
BL = B // NCORES      # 2 batch rows per core
T = BL * S            # 1024 tokens per core
HC = H // 128         # 6 hidden chunks
FFC = FF // 128       # 24 ffn chunks
EPS = 1e-5

f32 = mybir.dt.float32
f32r = mybir.dt.float32r
bf16 = mybir.dt.bfloat16
AF = mybir.ActivationFunctionType
AX = mybir.AxisListType
OP = mybir.AluOpType
ts = bass.ts

MAX_WAITS = 1

BF = ml_dtypes.bfloat16


class PatchedTileContext(TileContext):
    """Workaround for this walrus build's 1-sync-wait-per-instruction limit:
    split excess semaphore waits onto single-wait NOPs inserted immediately
    before the owning instruction (same engine, same program point)."""

    def _split_excess_waits(self, ordered):
        nc = self.nc
        for bb_name, insts in list(ordered.items()):
            new_list = []
            changed = False
            for inst in insts:
                si = getattr(inst, "sync_info", None)
                if si is not None and len(si.on_wait) > MAX_WAITS:
                    waits = list(si.on_wait)
                    movable = [
                        w for w in waits
                        if w.sync_type == "semaphore" and w.wait_mode == "sem-ge-imm"
                    ]
                    n_fixed = len(waits) - len(movable)
                    keep_n = max(0, MAX_WAITS - n_fixed)
                    n_over = max(0, len(movable) - keep_n)
                    overflow = movable[:n_over]
                    keep = [w for w in waits if w not in overflow]
                    assert len(keep) <= MAX_WAITS, (
                        f"cannot legalize waits on {inst.name}"
                    )
                    for w in overflow:
                        nop = mybir.InstNoOp(
                            name=f"I-{nc.next_id()}",
                            sync_info=mybir.SyncInfo(on_wait=[w], on_update=[]),
                            bass_nofuse=True,
                            engine=inst.engine,
                        )
                        new_list.append(nop)
                    inst.sync_info = mybir.SyncInfo(
                        on_wait=keep, on_update=list(si.on_update)
                    )
                    changed = True
                new_list.append(inst)
            if changed:
                ordered[bb_name] = new_list

    def _lower_ordered_insts(self, ordered):
        self._split_excess_waits(ordered)
        return super()._lower_ordered_insts(ordered)

    def _drain_and_barrier(self, tick_clock, wait_clock):
        nops = [self.nc.sync.nop(nofuse=True, hint=f"dw_{i}") for i in range(40)]
        drain_inst = self.nc.sync.drain()
        wait_clock.add_sem_waits(
            drain_inst.ins, ScopedClock({None: tick_clock.global_clock})
        )
        si = drain_inst.ins.sync_info
        if si is not None and len(si.on_wait) > 1:
            waits = list(si.on_wait)
            rest, keep = waits[:-1], waits[-1:]
            assert len(rest) <= len(nops)
            for nop_bi, w in zip(nops, rest):
                nop_bi.ins.sync_info = mybir.SyncInfo(on_wait=[w], on_update=[])
            drain_inst.ins.sync_info = mybir.SyncInfo(
                on_wait=keep, on_update=list(si.on_update)
            )
        self.nc.all_engine_barrier()
        assert self.sems is not None
        popped = self.nc._tile_sem_poison_stack.pop()
        assert popped is self._sem_poison
        self.nc.clear_and_free_semaphores(list(self.sems.allocated().values()))
        self.nc.all_engine_barrier()


def _r(ap):
    return ap.bitcast(f32r)


def _act_raw(nc, out, in_, func, bias=None):
    """scalar.activation without bass's Reciprocal/Rsqrt accuracy guard.
    out = func(in_ + bias); scale=1."""
    if bias is None:
        bias = nc.const_aps.scalar_like(0.0, in_)
    eng = nc.scalar
    ins = [eng.lower_ap(in_), eng.lower_ap(bias),
           mybir.ImmediateValue(dtype=f32, value=1.0),
           mybir.ImmediateValue(dtype=f32, value=0.0)]
    return eng.add_instruction(
        mybir.InstActivation(
            name=nc.get_next_instruction_name(),
            func=func,
            ins=ins,
            outs=[eng.lower_ap(out)],
        )
    )


def build_program(n_layers=L, debug=False):
    nc = bass.Bass()

    x0T_d = nc.dram_tensor("x0T", [128, HC, T], bf16, kind="ExternalInput")
    wqkv_d = nc.dram_tensor("wqkvT", [n_layers, 128, HC, 3 * H], bf16,
                            kind="ExternalInput")
    wo_d = nc.dram_tensor("woT", [n_layers, HD, NH, H], bf16,
                          kind="ExternalInput")
    w1_d = nc.dram_tensor("w1T", [n_layers, 128, HC, FF], bf16,
                          kind="ExternalInput")
    w2_d = nc.dram_tensor("w2T", [n_layers, 128, FFC, H], bf16,
                          kind="ExternalInput")
    wr_d = nc.dram_tensor("wrT", [128, HC, E], bf16, kind="ExternalInput")
    we1_d = nc.dram_tensor("we1T", [128, HC, FE], bf16, kind="ExternalInput")
    we2_d = nc.dram_tensor("we2T", [128, FFC, C], bf16, kind="ExternalInput")
    maske_d = nc.dram_tensor("maske", [B, E], f32, kind="ExternalInput")
    ones_d = nc.dram_tensor("ones", [128, 128], f32, kind="ExternalInput")
    onesb_d = nc.dram_tensor("onesb", [128, 128], bf16, kind="ExternalInput")
    id128_d = nc.dram_tensor("id128", [128, 128], f32, kind="ExternalInput")
    id16_d = nc.dram_tensor("id16", [16, 16], f32, kind="ExternalInput")
    y_d = nc.dram_tensor("y", [B, C], f32, kind="ExternalOutput")
    cc_in = nc.dram_tensor("cc_in", [BL, H], f32)
    cc_out = nc.dram_tensor("cc_out", [B, H], f32, addr_space="Shared")

    lp = nc.allow_low_precision(reason="bf16 activations/weights by design")
    lp.__enter__()
    with PatchedTileContext(nc) as tc:
        with tc.tile_pool(name="const", bufs=1) as cpool, \
             tc.tile_pool(name="act", bufs=1) as act, \
             tc.tile_pool(name="rows", bufs=2) as rows, \
             tc.tile_pool(name="psmm", bufs=4, space="PSUM") as psmm, \
             tc.tile_pool(name="pspo", bufs=2, space="PSUM") as pspo, \
             tc.tile_pool(name="psst", bufs=2, space="PSUM") as psst:

            # ---------------- constants
            onescol = cpool.tile([128, 1], bf16, tag="onescol")
            nc.sync.dma_start(onescol[:], onesb_d[:, 0:1])
            onesrow = cpool.tile([1, 128], bf16, tag="onesrow")
            nc.sync.dma_start(onesrow[:], onesb_d[0:1, :])
            id16 = cpool.tile([16, 16], f32, tag="id16")
            nc.sync.dma_start(id16[:], id16_d[:])
            id128 = cpool.tile([128, 128], f32, tag="id128")
            nc.sync.dma_start(id128[:], id128_d[:])
            ones128b = cpool.tile([128, 128], bf16, tag="ones128b")
            nc.sync.dma_start(ones128b[:], onesb_d[:])
            eps_row = cpool.tile([1, 1], f32, tag="eps_row")
            nc.vector.memset(eps_row[:], EPS)

            # ---------------- persistent activations (bf16)
            x = act.tile([128, HC, T], bf16, tag="x")
            nc.sync.dma_start(x[:, :, 0:512], x0T_d[:, :, 0:512])
            nc.sync.dma_start(x[:, :, 512:1024], x0T_d[:, :, 512:1024])
            hT = act.tile([128, HC, T], bf16, tag="hT")
            hT2 = act.tile([128, HC, T], bf16, tag="hT2")
            qT = act.tile([HD, NH, 512], bf16, tag="qT")
            kT = act.tile([HD, NH, 512], bf16, tag="kT")
            oT = act.tile([HD, NH, 512], bf16, tag="oT")
            pooledT = act.tile([128, HC, BL], f32, tag="pooledT")
            pool_tok = act.tile([BL, H], f32, tag="pool_tok")

            def layer_norm_half(bca, xsrc, hdst, tq, name):
                """hdst[:, :, tq*512:+512] = LN(xsrc same region). Stats via
                PE ones-matmuls, chain on DVE/scalar, broadcasts via PE."""
                tqs = ts(tq, 512)
                # s1 in array col-group 0, s2 in col-group 1: the pair
                # runs concurrently in the PE (separate output strips/XBUS)
                s1 = psst.tile([1, 512], f32, tag="stat", name=f"s1_{name}")
                s2t = psst.tile([33, 512], f32, tag="stat",
                                name=f"s2_{name}")
                s2 = s2t[32:33, :]
                for hc in range(HC):
                    sq = bca.tile([128, 512], bf16, tag="sq",
                                  name=f"sq_{name}_{hc}")
                    nc.scalar.activation(sq[:], xsrc[:, hc, tqs], AF.Square)
                    nc.tensor.matmul(s1[:], onescol[:], xsrc[:, hc, tqs],
                                     start=(hc == 0), stop=(hc == HC - 1),
                                     tile_position=(0, 0))
                    nc.tensor.matmul(s2, onescol[:], sq[:],
                                     start=(hc == 0), stop=(hc == HC - 1),
                                     tile_position=(0, 32))
                mu = rows.tile([1, 512], f32, tag="mu", name=f"mu_{name}")
                var = rows.tile([1, 512], f32, tag="var", name=f"var_{name}")
                rstd = rows.tile([1, 512], f32, tag="rstd",
                                 name=f"rstd_{name}")
                nbr = rows.tile([1, 512], f32, tag="nbr", name=f"nbr_{name}")
                msq = rows.tile([1, 512], f32, tag="msq", name=f"msq_{name}")
                nc.vector.tensor_scalar_mul(mu[:], s1[:], 1.0 / H)
                nc.vector.tensor_scalar_mul(var[:], s2[:], 1.0 / H)
                nc.vector.tensor_tensor(msq[:], mu[:], mu[:], OP.mult)
                nc.vector.tensor_tensor(var[:], var[:], msq[:], OP.subtract)
                _act_raw(nc, rstd[:], var[:], AF.Rsqrt, bias=eps_row[:])
                # nbr = -mu * rstd  (so hT = x*rb + nbr_bcast)
                nc.vector.tensor_tensor(nbr[:], mu[:], rstd[:], OP.mult)
                nbr_b = rows.tile([1, 512], bf16, tag="nbr_b",
                                  name=f"nbr_b_{name}")
                rstd_b = rows.tile([1, 512], bf16, tag="rstd_b",
                                   name=f"rstd_b_{name}")
                nc.vector.tensor_scalar_mul(nbr_b[:], nbr[:], -1.0)
                nc.scalar.copy(rstd_b[:], rstd[:])
                prb = psmm.tile([128, 512], f32, tag="mm", name=f"rb_{name}")
                pnb = psmm.tile([128, 512], f32, tag="mm", name=f"nb_{name}")
                nc.tensor.matmul(prb[:], onesrow[:], rstd_b[:],
                                 start=True, stop=True)
                nc.tensor.matmul(pnb[:], onesrow[:], nbr_b[:],
                                 start=True, stop=True)
                rb_b = bca.tile([128, 512], bf16, tag="rb_b",
                                name=f"rb_b_{name}")
                nb_b = bca.tile([128, 512], bf16, tag="nb_b",
                                name=f"nb_b_{name}")
                nc.scalar.copy(rb_b[:], prb[:])
                nc.scalar.copy(nb_b[:], pnb[:])
                for hc in range(HC):
                    t1 = bca.tile([128, 512], bf16, tag="lnt",
                                  name=f"lnt_{name}_{hc}")
                    nc.vector.tensor_tensor(t1[:], xsrc[:, hc, tqs], rb_b[:],
                                            OP.mult)
                    nc.vector.tensor_tensor(hdst[:, hc, tqs], t1[:], nb_b[:],
                                            OP.add)

            with tc.tile_pool(name="bca", bufs=2) as bca, \
                 tc.tile_pool(name="attn", bufs=2) as attn, \
                 tc.tile_pool(name="fft", bufs=3) as fftp, \
                 tc.tile_pool(name="wqkv", bufs=1) as wqkvp, \
                 tc.tile_pool(name="wo", bufs=1) as wop, \
                 tc.tile_pool(name="w1", bufs=2) as w1p, \
                 tc.tile_pool(name="w2", bufs=2) as w2p:

                # LN1 for layer 0 (later layers' LN1 is emitted inside
                # the previous layer's FFN so it hides under the other half)
                for tq in range(2):
                    layer_norm_half(bca, x, hT, tq, f"ln1_0_{tq}")

                for l in range(n_layers):
                    # ---------------- layer weights (1 DMA each for qkv/wo)
                    wqkv_sb = wqkvp.tile([128, HC, 3 * H], bf16, tag="wqkv",
                                         name=f"wqkv_{l}")
                    nc.sync.dma_start(wqkv_sb[:], wqkv_d[l])
                    wo_sb = wop.tile([HD, NH, H], bf16, tag="wo",
                                     name=f"wo_{l}")
                    nc.sync.dma_start(wo_sb[:], wo_d[l])

                    # ---------------- attention per batch row (=token half)
                    for b2 in range(BL):
                        tqs = ts(b2, 512)
                        # QKV projections
                        # 128-wide weight slices (not 96) so FWL stays
                        # enabled; rows 96-127 of the psum are next-head
                        # garbage, dropped at evacuation
                        for h in range(NH):
                            pq = psmm.tile([128, 512], f32, tag="mm",
                                           name=f"pq_{l}_{b2}_{h}")
                            pk = psmm.tile([128, 512], f32, tag="mm",
                                           name=f"pk_{l}_{b2}_{h}")
                            for hc in range(HC):
                                rhs = hT[:, hc, tqs]
                                nc.tensor.matmul(
                                    pq[:], wqkv_sb[:, hc, h * HD:h * HD + 128],
                                    rhs, start=(hc == 0), stop=(hc == HC - 1))
                                nc.tensor.matmul(
                                    pk[:],
                                    wqkv_sb[:, hc,
                                            H + h * HD:H + h * HD + 128],
                                    rhs, start=(hc == 0), stop=(hc == HC - 1))
                            nc.any.tensor_copy(qT[:, h, :], pq[:HD, :])
                            nc.any.tensor_copy(kT[:, h, :], pk[:HD, :])
                        # V (token-major, ones-augmented for softmax denom)
                        v_augf = attn.tile([128, 4 * NH * (HD + 1) + 32],
                                           bf16, tag="vaug",
                                           name=f"vaug_{l}_{b2}")
                        v_aug = v_augf[:, :4 * NH * (HD + 1)].rearrange(
                            "p (tk h d) -> p tk h d", tk=4, h=NH)
                        nc.vector.memset(v_aug[:, :, :, HD:], 1.0)
                        nc.vector.memset(
                            v_augf[:, 4 * NH * (HD + 1):], 0.0)
                        for n2 in range(2):
                            for tt in range(4):
                                pv = psmm.tile([128, 384], f32, tag="mm",
                                               name=f"pv_{l}_{b2}_{n2}_{tt}")
                                for hc in range(HC):
                                    lhs = hT[:, hc, b2 * 512 + tt * 128:
                                             b2 * 512 + (tt + 1) * 128]
                                    nc.tensor.matmul(
                                        pv[:], lhs,
                                        wqkv_sb[:, hc,
                                                2 * H + n2 * 384:
                                                2 * H + (n2 + 1) * 384],
                                        start=(hc == 0), stop=(hc == HC - 1))
                                dst = v_aug[:, tt, n2 * 4:(n2 + 1) * 4, :HD]
                                nc.any.tensor_copy(
                                    dst,
                                    pv[:].rearrange("p (h d) -> p h d", h=4))
                        # scores -> softmax -> AV for all heads; denominators
                        # batched 4 heads per DVE reciprocal (rows 0/32/64/96)
                        dns = []
                        rcps = []
                        for hb in range(2):
                            dn = rows.tile([128, 512], f32, tag="dn",
                                           name=f"dn_{l}_{b2}_{hb}")
                            nc.vector.memset(dn[:], 1.0)
                            dns.append(dn)
                        for h in range(NH):
                            hb, h4 = divmod(h, 4)
                            expT = attn.tile([128, 4, 512], bf16,
                                             tag="expT",
                                             name=f"expT_{l}_{b2}_{h}")
                            for tk in range(4):
                                psc = psmm.tile(
                                    [128, 512], f32, tag="mm",
                                    name=f"sc_{l}_{b2}_{h}_{tk}")
                                nc.tensor.matmul(
                                    psc[:], kT[:, h, ts(tk, 128)],
                                    qT[:, h, :], start=True, stop=True)
                                nc.scalar.activation(
                                    expT[:, tk, :], psc[:], AF.Exp,
                                    scale=float(1.0 / np.sqrt(HD)))
                            po = pspo.tile([128, 512], f32, tag="po",
                                           name=f"po_{l}_{b2}_{h}")
                            for tk in range(4):
                                off = (tk * NH + h) * (HD + 1)
                                nc.tensor.matmul(po[:],
                                                 v_augf[:, off:off + 128],
                                                 expT[:, tk, :],
                                                 start=(tk == 0),
                                                 stop=(tk == 3))
                            nc.any.tensor_copy(oT[:, h, :], po[:HD, :])
                            nc.any.tensor_copy(
                                dns[hb][32 * h4:32 * h4 + 1, :],
                                po[HD:HD + 1, :])
                            if h4 == 3:
                                rcp_bf = rows.tile([128, 512], bf16,
                                                   tag="rcp_bf",
                                                   name=f"rcpbf_{l}_{b2}_{hb}")
                                nc.vector.reciprocal(rcp_bf[:], dns[hb][:])
                                rcps.append(rcp_bf)
                        # normalize (prb tiles allocated after all psc tiles
                        # so blocked broadcasts don't plug the mm rotation)
                        for h in range(NH):
                            hb, h4 = divmod(h, 4)
                            prb = psmm.tile([128, 512], f32, tag="mm",
                                            name=f"prb_{l}_{b2}_{h}")
                            nc.tensor.matmul(
                                prb[:],
                                ones128b[32 * h4:32 * h4 + 1, :],
                                rcps[hb][32 * h4:32 * h4 + 1, :],
                                start=True, stop=True,
                                tile_position=(32 * h4, 0))
                            nc.vector.tensor_tensor(oT[:, h, :],
                                                    oT[:, h, :],
                                                    prb[:HD, :], OP.mult)
                        # Wo + residual
                        for m in range(HC):
                            pwo = pspo.tile([128, 512], f32, tag="po",
                                            name=f"pwo_{l}_{b2}_{m}")
                            for h in range(NH):
                                nc.tensor.matmul(pwo[:],
                                                 wo_sb[:, h, ts(m, 128)],
                                                 oT[:, h, :],
                                                 start=(h == 0),
                                                 stop=(h == NH - 1))
                            nc.vector.tensor_tensor(x[:, m, tqs],
                                                    x[:, m, tqs], pwo[:],
                                                    OP.add)
                        # LN2 for this half right away (overlaps other half's
                        # attention)
                        layer_norm_half(bca, x, hT2, b2, f"ln2_{l}_{b2}")

                    # ---------------- FFN per token half; each half's output
                    # feeds the next layer's LN1 (or the final LN) immediately
                    for tq in range(2):
                        tqs = ts(tq, 512)
                        ffT_tiles = {}
                        for fg in range(6):
                            w1g = w1p.tile([128, HC, 512], bf16, tag="w1",
                                           name=f"w1_{l}_{tq}_{fg}")
                            nc.sync.dma_start(
                                w1g[:], w1_d[l, :, :, ts(fg, 512)])
                            ffT = fftp.tile([128, 4, 512], bf16, tag="ffT",
                                            name=f"ffT_{l}_{tq}_{fg}")
                            ffT_tiles[(tq, fg)] = ffT
                            for ff in range(4):
                                pf = psmm.tile([128, 512], f32, tag="mm",
                                               name=f"pf_{l}_{tq}_{fg}_{ff}")
                                for hc in range(HC):
                                    nc.tensor.matmul(
                                        pf[:],
                                        w1g[:, hc, ts(ff, 128)],
                                        hT2[:, hc, tqs],
                                        start=(hc == 0), stop=(hc == HC - 1))
                                nc.scalar.activation(ffT[:, ff, :], pf[:],
                                                     AF.Gelu)
                        for fg in range(6):
                            w2g = w2p.tile([128, 4, H], bf16, tag="w2",
                                           name=f"w2_{l}_{tq}_{fg}")
                            nc.sync.dma_start(
                                w2g[:], w2_d[l, :, fg * 4:(fg + 1) * 4, :])
                            ffTg = ffT_tiles[(tq, fg)]
                            for m in range(HC):
                                px = pspo.tile([128, 512], f32, tag="po",
                                               name=f"px_{l}_{tq}_{fg}_{m}")
                                for ff in range(4):
                                    nc.tensor.matmul(
                                        px[:], w2g[:, ff, ts(m, 128)],
                                        ffTg[:, ff, :],
                                        start=(ff == 0), stop=(ff == 3))
                                nc.vector.tensor_tensor(x[:, m, tqs],
                                                        x[:, m, tqs], px[:],
                                                        OP.add)
                        if l < n_layers - 1:
                            layer_norm_half(bca, x, hT, tq,
                                            f"ln1_{l + 1}_{tq}")
                        else:
                            layer_norm_half(bca, x, hT, tq, f"lnf_{tq}")
                            acc = rows.tile([128, HC, 1], f32, tag="poolacc",
                                            name=f"poolacc_{tq}")
                            nc.vector.reduce_sum(acc[:],
                                                 hT[:, :, ts(tq, 512)],
                                                 axis=AX.X)
                            nc.vector.tensor_scalar_mul(
                                pooledT[:, :, tq:tq + 1], acc[:], 1.0 / S)

                # -------------- pooled transpose + AllGather
                for hc in range(HC):
                    pt = psmm.tile([BL, 128], f32, tag="mm",
                                   name=f"ptr_{hc}")
                    nc.tensor.transpose(pt[:], pooledT[:, hc, :], id128[:])
                    nc.any.tensor_copy(pool_tok[:, ts(hc, 128)], pt[:])
                nc.gpsimd.dma_start(cc_in[:], pool_tok[:])
                nc.gpsimd.collective_compute(
                    "AllGather", OP.bypass,
                    replica_groups=[list(range(NCORES))],
                    ins=[cc_in[:]], outs=[cc_out[:]],
                )

            # -------------- MoE head (expert-parallel).  The backbone
            # weight/attn pools are closed, so this pool reuses their SBUF and
            # the expert-weight DMAs start as soon as the last readers retire.
            with tc.tile_pool(name="head", bufs=1) as hp:
                we1_sb = hp.tile([128, HC, FE], bf16, tag="we1")
                nc.sync.dma_start(we1_sb[:], we1_d[:])
                we2_sb = hp.tile([128, FFC, C], bf16, tag="we2")
                nc.sync.dma_start(we2_sb[:], we2_d[:])
                wr_sb = hp.tile([128, HC, E], bf16, tag="wr")
                nc.sync.dma_start(wr_sb[:], wr_d[:])
                maske = hp.tile([B, E], f32, tag="maske")
                nc.sync.dma_start(maske[:], maske_d[:])
                id16b = hp.tile([16, 16], bf16, tag="id16b")
                nc.any.tensor_copy(id16b[:], id16[:])

                pg = hp.tile([B, H], f32, tag="pg")
                nc.gpsimd.dma_start(pg[:], cc_out[:])
                paT = hp.tile([128, HC, B], bf16, tag="paT")
                for hc in range(HC):
                    ptr = psmm.tile([128, B], f32, tag="mm",
                                    name=f"hptr_{hc}")
                    nc.tensor.transpose(ptr[:], pg[:, ts(hc, 128)], id16[:])
                    nc.any.tensor_copy(paT[:, hc, :], ptr[:])
                # gate
                pgl = psst.tile([B, E], f32, tag="stat", name="pgl")
                for hc in range(HC):
                    nc.tensor.matmul(pgl[:], paT[:, hc, :], wr_sb[:, hc, :],
                                     start=(hc == 0), stop=(hc == HC - 1))
                gate = hp.tile([B, E], f32, tag="gate")
                gmax = rows.tile([B, 1], f32, tag="grow")
                nc.vector.reduce_max(gmax[:], pgl[:], axis=AX.X)
                ngmax = rows.tile([B, 1], f32, tag="grow2")
                nc.vector.tensor_scalar_mul(ngmax[:], gmax[:], -1.0)
                nc.scalar.activation(gate[:], pgl[:], AF.Exp, bias=ngmax[:])
                gsum = rows.tile([B, 1], f32, tag="grow3")
                nc.vector.reduce_sum(gsum[:], gate[:], axis=AX.X)
                grecip = rows.tile([B, 1], f32, tag="grow4")
                nc.vector.reciprocal(grecip[:], gsum[:])
                nc.vector.tensor_scalar_mul(gate[:], gate[:], grecip[:])
                gcol = hp.tile([B, 1], f32, tag="gcol")
                nc.vector.tensor_tensor(maske[:], gate[:], maske[:], OP.mult)
                nc.vector.reduce_sum(gcol[:], maske[:], axis=AX.X)

                # expert hidden, token-major [B, FE]
                eh_tok = hp.tile([B, FE], bf16, tag="eh_tok")
                for fet in range(6):
                    pe_ = psmm.tile([B, 512], f32, tag="mm",
                                    name=f"pe_{fet}")
                    for hc in range(HC):
                        nc.tensor.matmul(pe_[:], paT[:, hc, :],
                                         we1_sb[:, hc, ts(fet, 512)],
                                         start=(hc == 0), stop=(hc == HC - 1))
                    nc.scalar.activation(eh_tok[:, ts(fet, 512)], pe_[:],
                                         AF.Gelu)
                # transpose to feature-major [FE, B]
                ehT = hp.tile([128, FFC, B], bf16, tag="ehT")
                for fc in range(FFC):
                    ptb = psmm.tile([128, B], bf16, tag="mm",
                                    name=f"ptb_{fc}")
                    nc.tensor.transpose(ptb[:], eh_tok[:, ts(fc, 128)],
                                        id16b[:])
                    nc.any.tensor_copy(ehT[:, fc, :], ptb[:])
                # expert logits, scaled by this expert's gate column
                y_sb = hp.tile([B, C], f32, tag="y")
                for cn in range(2):
                    csz = C // 2
                    pel = pspo.tile([B, csz], f32, tag="po",
                                    name=f"pel_{cn}")
                    for fc in range(FFC):
                        nc.tensor.matmul(pel[:], ehT[:, fc, :],
                                         we2_sb[:, fc, ts(cn, csz)],
                                         start=(fc == 0), stop=(fc == FFC - 1))
                    nc.vector.tensor_scalar_mul(y_sb[:, ts(cn, csz)], pel[:],
                                                gcol[:])
                nc.sync.dma_start(y_d[:], y_sb[:])

    lp.__exit__(None, None, None)
    return nc, {}


_CACHE = {}


def _get_program(n_layers=L, debug=False):
    key = (n_layers, debug)
    if key not in _CACHE:
        _CACHE[key] = build_program(n_layers, debug)
    return _CACHE[key]


def prepare_inputs(inputs, n_layers=L):
    """Host-side shard prep: embedding gather, bf16 weight transposes,
    per-core slicing, asserts."""
    ids = np.asarray(inputs["input_ids"])
    mask = np.asarray(inputs["attention_mask"])
    assert (mask == 1).all(), "kernel assumes attention_mask == ones"
    for k in ("bqkv", "bo", "b1", "b2", "br", "be1", "be2",
              "ln1_b", "ln2_b", "lnf_b"):
        assert not np.any(np.asarray(inputs[k])), f"{k} must be zero"
    for k in ("ln1_g", "ln2_g", "lnf_g"):
        assert np.all(np.asarray(inputs[k]) == 1.0), f"{k} must be ones"

    tok = np.asarray(inputs["tok_emb"], np.float32)
    pos = np.asarray(inputs["pos_emb"], np.float32)
    x0 = tok[ids] + pos[None]                      # [B, S, H]

    wqkv = np.asarray(inputs["Wqkv"], np.float32)[:n_layers]   # [L,H,3H]
    wqkvT = np.ascontiguousarray(
        wqkv.reshape(n_layers, HC, 128, 3 * H).transpose(0, 2, 1, 3)
    ).astype(BF)                                               # [L,128,HC,3H]
    wo = np.asarray(inputs["Wo"], np.float32)[:n_layers]       # [L,H,H]
    woT = np.ascontiguousarray(
        wo.reshape(n_layers, NH, HD, H).transpose(0, 2, 1, 3)
    ).astype(BF)                                               # [L,HD,NH,H]
    w1 = np.asarray(inputs["W1"], np.float32)[:n_layers]       # [L,H,FF]
    w1T = np.ascontiguousarray(
        w1.reshape(n_layers, HC, 128, FF).transpose(0, 2, 1, 3)
    ).astype(BF)                                               # [L,128,HC,FF]
    w2 = np.asarray(inputs["W2"], np.float32)[:n_layers]       # [L,FF,H]
    w2T = np.ascontiguousarray(
        w2.reshape(n_layers, FFC, 128, H).transpose(0, 2, 1, 3)
    ).astype(BF)                                               # [L,128,FFC,H]
    wr = np.asarray(inputs["Wr"], np.float32)                  # [H,E]
    wrT = np.ascontiguousarray(
        wr.reshape(HC, 128, E).transpose(1, 0, 2)).astype(BF)  # [128,HC,E]
    we1 = np.asarray(inputs["We1"], np.float32)                # [E,H,FE]
    we2 = np.asarray(inputs["We2"], np.float32)                # [E,FE,C]
    id16 = np.eye(16, dtype=np.float32)
    id128 = np.eye(128, dtype=np.float32)
    ones = np.ones((128, 128), np.float32)

    in_maps = []
    for c in range(NCORES):
        rows_ = x0[c * BL:(c + 1) * BL]             # [BL, S, H]
        x0T = rows_.reshape(T, H).T                 # [H, T]
        x0Tr = np.ascontiguousarray(
            x0T.reshape(HC, 128, T).transpose(1, 0, 2)).astype(BF)
        maske = np.zeros((B, E), np.float32)
        maske[:, c] = 1.0
        we1T = np.ascontiguousarray(
            we1[c].reshape(HC, 128, FE).transpose(1, 0, 2)).astype(BF)
        we2T = np.ascontiguousarray(
            we2[c].reshape(FFC, 128, C).transpose(1, 0, 2)).astype(BF)
        in_maps.append({
            "x0T": x0Tr, "wqkvT": wqkvT, "woT": woT, "w1T": w1T, "w2T": w2T,
            "wrT": wrT, "we1T": we1T, "we2T": we2T,
            "maske": maske, "ones": ones,
            "onesb": ones.astype(BF), "id128": id128, "id16": id16,
        })
    return in_maps


def kernel(**inputs):
    nc, _dbg = _get_program(L, debug=False)
    in_maps = prepare_inputs(inputs, L)
    res = run_bass_kernel_spmd(nc, in_maps, core_ids=list(range(NCORES)))
    out = np.zeros((B, C), np.float32)
    for r_ in res.results:
        out += r_["y"]
    return out
